# revision 1
# baseline (speedup 1.0000x reference)
"""Dense 3-layer GAT on 8 TRN2 NeuronCores.

Sharding: each core owns 512 query nodes (rows of the attention score
matrix). Per layer, each core computes h = x @ W and f = x @ (W @ a) for
its own nodes, AllGathers h (bf16) and f (f32) across the 8 cores, then
computes its 512-query slab of masked-softmax attention and the attended
output.

Everything on device is kept transposed (features on partitions, nodes
on the free dim) so layer outputs feed the next layer's matmuls with no
transposes. The h/attention path runs in bf16 (TensorE streams bf16 at
2x the fp32 rate and DVE hits its packed modes); the attention-logit
path (f = x @ (W@a), leaky-relu bias, exp input) stays fp32 since the
softmax is sensitive to absolute errors in the logits.

kernel(**inputs) takes the full unsharded inputs and returns the full
[4096, 256] output.
"""

from contextlib import ExitStack

import numpy as np
import ml_dtypes

import concourse.mybir as mybir
import concourse.tile as tile
from concourse import bacc
from concourse.bass_utils import run_bass_kernel_spmd
from concourse.masks import make_identity

P = 128
N_NODES = 4096
S = 512                    # nodes per core
NB = N_NODES // P          # 32 global key blocks
H = 4
LAYERS = [(512, 512), (2048, 512), (2048, 64)]
F32 = mybir.dt.float32
BF16 = mybir.dt.bfloat16
AF = mybir.ActivationFunctionType
ALU = mybir.AluOpType

_CACHE = {}


def _build():
    nc = bacc.Bacc("TRN2", target_bir_lowering=False, debug=False, num_devices=8)

    xT0_d = nc.dram_tensor("xT0", [512, S], F32, kind="ExternalInput")
    adjT_d = nc.dram_tensor("adjT", [N_NODES, S], BF16, kind="ExternalInput")
    W_d = []
    WA_d = []
    for li, (fin, fout) in enumerate(LAYERS):
        W_d.append(nc.dram_tensor(f"W{li}", [H, fin, fout], BF16, kind="ExternalInput"))
        WA_d.append(nc.dram_tensor(f"WA{li}", [fin, 2 * H], F32, kind="ExternalInput"))
    outT_d = nc.dram_tensor("outT", [H * 64, S], F32, kind="ExternalOutput")

    with tile.TileContext(nc) as tc:
        with ExitStack() as ctx:
            constp = ctx.enter_context(tc.tile_pool(name="const", bufs=1))
            adjp = ctx.enter_context(tc.tile_pool(name="adjp", bufs=1))
            xtp = ctx.enter_context(tc.tile_pool(name="xt", bufs=20))
            xbp = ctx.enter_context(tc.tile_pool(name="xb", bufs=21))
            wtp = ctx.enter_context(tc.tile_pool(name="wt", bufs=17))
            wap = ctx.enter_context(tc.tile_pool(name="wap", bufs=17))
            hfp = ctx.enter_context(tc.tile_pool(name="hfp", bufs=3))
            hgp = ctx.enter_context(tc.tile_pool(name="hgp", bufs=4))
            scp = ctx.enter_context(tc.tile_pool(name="scp", bufs=2))
            fbp = ctx.enter_context(tc.tile_pool(name="fbp", bufs=1))
            fdp = ctx.enter_context(tc.tile_pool(name="fdp", bufs=2))
            frp = ctx.enter_context(tc.tile_pool(name="frp", bufs=3))
            evp = ctx.enter_context(tc.tile_pool(name="evp", bufs=2))
            rcp = ctx.enter_context(tc.tile_pool(name="rcp", bufs=2))
            prp = ctx.enter_context(tc.tile_pool(name="prp", bufs=6))
            psA = ctx.enter_context(tc.tile_pool(name="psA", bufs=2, space="PSUM"))
            psO = ctx.enter_context(tc.tile_pool(name="psO", bufs=4, space="PSUM"))
            psR = ctx.enter_context(tc.tile_pool(name="psR", bufs=1, space="PSUM"))
            dr = ctx.enter_context(tc.tile_pool(name="dram", bufs=1, space="DRAM"))

            ident = constp.tile([P, P], F32, tag="ident")
            make_identity(nc, ident[:])
            # dummy collective to absorb ncfw first-call staging cost while
            # the layer-0 matmuls run
            warm_in = dr.tile([P, 4], F32, tag="warm_in")
            warm_out = dr.tile([8, P, 4], F32, tag="warm_out", addr_space="Shared")
            warm_sb = constp.tile([P, 4], F32, tag="warm_sb", name="warm_sb")
            nc.any.memset(warm_sb[:], 0.0)
            nc.sync.dma_start(warm_in[:], warm_sb[:])
            nc.gpsimd.collective_compute(
                "AllGather", ALU.bypass,
                replica_groups=[list(range(8))],
                ins=[warm_in[:].opt()], outs=[warm_out[:].opt()],
            )
            ones_r = constp.tile([1, P], F32, tag="ones_r")
            nc.any.memset(ones_r[:], 1.0)
            ones_c = constp.tile([P, 1], BF16, tag="ones_c")
            nc.any.memset(ones_c[:], 1.0)

            # resident adjacency (transposed slab), bf16, [key m, own query n]
            adjT_res = adjp.tile([P, NB, S], BF16, tag="adjT")
            nc.sync.dma_start(
                adjT_res[:], adjT_d[:].rearrange("(nb p) n -> p nb n", p=P)
            )

            # layer-0 x^T (own nodes): f32 for the f matmuls + bf16 for h
            xt_cur = []
            xb_cur = []
            for kb in range(4):
                t = xtp.tile([P, S], F32, tag="xt")
                nc.sync.dma_start(t[:], xT0_d[kb * P:(kb + 1) * P, :])
                xt_cur.append(t)
                tb = xbp.tile([P, S], BF16, tag="xb")
                nc.vector.tensor_copy(tb[:], t[:])
                xb_cur.append(tb)

            for li, (fin, fout) in enumerate(LAYERS):
                KB = fin // P
                agh_in = dr.tile([H, S, fout], BF16, tag=f"aghi{li}")
                agh_out = [
                    dr.tile([8, 2, S, fout], BF16, tag=f"agho{li}_{pp}",
                            name=f"agho{li}_{pp}", addr_space="Shared")
                    for pp in range(2)
                ]
                agf_in = dr.tile([2 * H, S], F32, tag=f"agfi{li}")
                agf_out = dr.tile([8, 2 * H, S], F32, tag=f"agfo{li}",
                                  addr_space="Shared")

                # ---- phase A1: f = x @ WA (fp32) ----
                wa_tiles = []
                for kb in range(KB):
                    t = wap.tile([P, 2 * H], F32, tag="wa")
                    nc.sync.dma_start(t[:], WA_d[li][kb * P:(kb + 1) * P, :])
                    wa_tiles.append(t)
                ptf = psA.tile([P, S], F32, tag="ph", name=f"ptf{li}")
                for b in range(4):
                    pf = psA.tile([P, S], F32, tag="ph", name=f"pf{li}_{b}")
                    for kb in range(KB):
                        nc.tensor.matmul(
                            pf[:, 0:2 * H],
                            xt_cur[kb][:, b * P:(b + 1) * P],
                            wa_tiles[kb][:],
                            start=(kb == 0), stop=(kb == KB - 1),
                        )
                    f_sb = hfp.tile([P, 2 * H], F32, tag="fsb")
                    nc.vector.tensor_copy(f_sb[:], pf[:, 0:2 * H])
                    nc.tensor.transpose(
                        ptf[0:2 * H, b * P:(b + 1) * P], f_sb[:], ident[:]
                    )
                fT_loc = hfp.tile([2 * H, S], F32, tag="ftl")
                nc.scalar.copy(fT_loc[:], ptf[0:2 * H, :])
                nc.sync.dma_start(agf_in[:], fT_loc[:])

                # f gather is tiny; issue it before the h matmuls so it hides
                nc.gpsimd.collective_compute(
                    "AllGather", ALU.bypass,
                    replica_groups=[list(range(8))],
                    ins=[agf_in[:].opt()], outs=[agf_out[:].opt()],
                )

                # ---- phase A2: h = x @ W (bf16), shared-weight loop ----
                w_tiles = {}
                for h in range(H):
                    for kb in range(KB):
                        t = wtp.tile([P, fout], BF16, tag="wt", name=f"w{li}_{h}_{kb}")
                        nc.sync.dma_start(t[:], W_d[li][h, kb * P:(kb + 1) * P, :])
                        w_tiles[(h, kb)] = t
                for h in range(H):
                    for b in range(4):
                        ph = psA.tile([P, S], F32, tag="ph", name=f"ph{li}_{h}_{b}")
                        for kb in range(KB):
                            nc.tensor.matmul(
                                ph[:, 0:fout],
                                xb_cur[kb][:, b * P:(b + 1) * P],
                                w_tiles[(h, kb)][:],
                                start=(kb == 0), stop=(kb == KB - 1),
                            )
                        h_sb = hfp.tile([P, fout], BF16, tag="hsb")
                        nc.vector.tensor_copy(h_sb[:], ph[:, 0:fout])
                        nc.gpsimd.dma_start(agh_in[h, b * P:(b + 1) * P, :], h_sb[:])
                    # per-head-pair gather overlaps the next heads' matmuls
                    if h % 2 == 1:
                        nc.gpsimd.collective_compute(
                            "AllGather", ALU.bypass,
                            replica_groups=[list(range(8))],
                            ins=[agh_in[h - 1:h + 1].opt()],
                            outs=[agh_out[h // 2][:].opt()],
                        )

                # ---- phase C: f_src broadcast + f_dst layout ----
                fsb_bcast = fbp.tile([P, H, S], F32, tag="fsb_b")
                for h in range(H):
                    fr = frp.tile([1, S], F32, tag="fr")
                    nc.sync.dma_start(fr[:], agf_in[2 * h:2 * h + 1, :])
                    pb = psA.tile([P, S], F32, tag="ph", name=f"pb{li}_{h}")
                    nc.tensor.matmul(pb[:], ones_r[:], fr[:], start=True, stop=True)
                    nc.scalar.copy(fsb_bcast[:, h, :], pb[:])
                fgat = fdp.tile([64, S], F32, tag="fgat")
                nc.sync.dma_start(fgat[:], agf_out[:].rearrange("r j m -> (r j) m"))
                ptd = psA.tile([P, S], F32, tag="ph", name=f"ptd{li}")
                for c in range(4):
                    nc.tensor.transpose(
                        ptd[:, c * 64:(c + 1) * 64],
                        fgat[:, c * P:(c + 1) * P],
                        ident[0:64, 0:64],
                    )
                # fT_sb[ml, mh, r, j] = f[j][r*512 + mh*128 + ml]
                fT_sb = fdp.tile([P, 4, 8, 2 * H], F32, tag="fdst")
                nc.scalar.copy(
                    fT_sb[:], ptd[:, 0:256].rearrange("p (mh rj) -> p mh rj", mh=4)
                    .rearrange("p mh (r j) -> p mh r j", r=8)
                )

                # ---- phase D: attention ----
                xt_next = []
                xb_next = []
                nob = 1 if fout == 64 else 4
                pending_evict = None
                for h in range(H):
                    po = [
                        psO.tile([P, S], F32, tag="po", name=f"po{li}_{h}_{ob}")
                        for ob in range(nob)
                    ]
                    prs = psR.tile([1, S], F32, tag="prs", name=f"prs{li}_{h}")
                    for mbg in range(NB // 2):
                        lr4 = scp.tile([P, 2, S], F32, tag="lr")
                        for i in range(2):
                            nc.scalar.activation(
                                lr4[:, i, :], fsb_bcast[:, h, :], AF.Prelu,
                                bias=fT_sb[:, (2 * mbg + i) % 4, (2 * mbg + i) // 4, 2 * h + 1:2 * h + 2],
                                scale=1.0, alpha=0.2,
                            )
                        ex4 = scp.tile([P, 2, S], BF16, tag="ex")
                        nc.scalar.activation(ex4[:], lr4[:], AF.Exp, bias=0.0, scale=1.0)
                        st4 = scp.tile([P, 2, S], BF16, tag="st")
                        nc.vector.tensor_tensor(
                            st4[:], ex4[:], adjT_res[:, 2 * mbg:2 * mbg + 2, :], ALU.mult
                        )
                        for i in range(2):
                            mb = 2 * mbg + i
                            r, bsub = mb // 4, mb % 4
                            s_t = st4[:, i, :]
                            hg = hgp.tile([P, fout], BF16, tag="hg")
                            nc.gpsimd.dma_start(
                                hg[:], agh_out[h // 2][r, h % 2, bsub * P:(bsub + 1) * P, :]
                            )
                            for ob in range(nob):
                                nc.tensor.matmul(
                                    po[ob][:, :] if fout != 64 else po[ob][0:64, :],
                                    hg[:, ob * P:(ob + 1) * P] if fout != 64 else hg[:],
                                    s_t,
                                    start=(mb == 0), stop=(mb == NB - 1),
                                )
                            nc.tensor.matmul(
                                prs[:], ones_c[:], s_t,
                                start=(mb == 0), stop=(mb == NB - 1),
                            )

                    # free the PSUM banks promptly (ACT copies), defer the
                    # DVE normalize+elu so the next head's score TTs are not
                    # queued behind a premature PE wait on the vector engine
                    if pending_evict is not None:
                        pending_evict()
                        pending_evict = None
                    rows = 64 if fout == 64 else P
                    praw = []
                    for ob in range(nob):
                        src = po[ob][0:64, :] if fout == 64 else po[ob][:]
                        pr_sb = prp.tile([rows, S], F32, tag="praw", name=f"praw{li}_{h}_{ob}")
                        nc.scalar.copy(pr_sb[:], src)
                        praw.append(pr_sb)
                    rsum = rcp.tile([1, S], F32, tag="rsum")
                    nc.scalar.copy(rsum[:], prs[:])
                    pb2 = psA.tile([P, S], F32, tag="ph", name=f"pb2{li}_{h}")
                    nc.tensor.matmul(pb2[:], ones_r[:], rsum[:], start=True, stop=True)
                    rb0 = rcp.tile([P, S], F32, tag="rb0")
                    nc.scalar.copy(rb0[:], pb2[:])

                    def _evict(praw=praw, rb0=rb0, li=li, rows=rows):
                        rb = rcp.tile([P, S], F32, tag="rb", name=f"rb{li}")
                        nc.vector.reciprocal_approx_fast(rb[:], rb0[:])
                        for pr_sb in praw:
                            t0 = evp.tile([rows, S], F32, tag="t0", name=f"t0{li}")
                            nc.vector.tensor_tensor(t0[:], pr_sb[:], rb[0:rows, :], ALU.mult)
                            # elu(x) = min(exp(x) - 1, relu(x))
                            em = evp.tile([rows, S], F32, tag="em", name=f"em{li}")
                            nc.scalar.activation(em[:], t0[:], AF.Exp, bias=0.0, scale=1.0)
                            rl = evp.tile([rows, S], F32, tag="rl", name=f"rl{li}")
                            nc.vector.tensor_scalar_max(rl[:], t0[:], 0.0)
                            xnt = xtp.tile([rows, S], F32, tag="xt", name=f"xt{li}")
                            nc.vector.scalar_tensor_tensor(
                                xnt[:], em[:], -1.0, rl[:], ALU.add, ALU.min
                            )
                            if li == 2:
                                em2 = evp.tile([rows, S], F32, tag="em", name=f"em2{li}")
                                nc.scalar.activation(em2[:], xnt[:], AF.Exp, bias=0.0, scale=1.0)
                                rl2 = evp.tile([rows, S], F32, tag="rl", name=f"rl2{li}")
                                nc.vector.tensor_scalar_max(rl2[:], xnt[:], 0.0)
                                x2 = xtp.tile([rows, S], F32, tag="xt", name=f"x2{li}")
                                nc.vector.scalar_tensor_tensor(
                                    x2[:], em2[:], -1.0, rl2[:], ALU.add, ALU.min
                                )
                                xnt = x2
                            xt_next.append(xnt)
                            if li < 2:
                                xbn = xbp.tile([rows, S], BF16, tag="xb", name=f"xb{li}")
                                nc.vector.tensor_copy(xbn[:], xnt[:])
                                xb_next.append(xbn)

                    pending_evict = _evict

                if pending_evict is not None:
                    pending_evict()
                    pending_evict = None

                xt_cur = xt_next
                xb_cur = xb_next

            # final output: xt_cur is 4 tiles of [64, 512] (head-major)
            for h in range(H):
                nc.sync.dma_start(outT_d[h * 64:(h + 1) * 64, :], xt_cur[h][:])

    nc.compile()
    return nc


def build_in_maps(inputs):
    node_feats = np.ascontiguousarray(inputs["node_feats"], dtype=np.float32)
    adj = np.asarray(inputs["adj"], dtype=np.float32)
    Ws = [np.asarray(inputs[f"W{i}"], dtype=np.float32) for i in range(3)]
    As = [np.asarray(inputs[f"a{i}"], dtype=np.float32) for i in range(3)]

    WAs = []
    for W, a in zip(Ws, As):
        wa = np.einsum(
            "hfo,hjo->fhj", W.astype(np.float64), a.astype(np.float64)
        ).reshape(W.shape[1], 2 * H).astype(np.float32)
        WAs.append(np.ascontiguousarray(wa))
    Wbf = [W.astype(ml_dtypes.bfloat16) for W in Ws]

    in_maps = []
    for c in range(8):
        rows = slice(c * S, (c + 1) * S)
        m = {
            "xT0": np.ascontiguousarray(node_feats[rows].T),
            "adjT": np.ascontiguousarray(adj[rows].T).astype(ml_dtypes.bfloat16),
        }
        for i in range(3):
            m[f"W{i}"] = Wbf[i]
            m[f"WA{i}"] = WAs[i]
        in_maps.append(m)
    return in_maps


def kernel(**inputs):
    if "nc" not in _CACHE:
        _CACHE["nc"] = _build()
    nc = _CACHE["nc"]
    in_maps = build_in_maps(inputs)
    res = run_bass_kernel_spmd(nc, in_maps, core_ids=list(range(8)))
    out = np.concatenate([r["outT"].T for r in res.results], axis=0)
    return np.ascontiguousarray(out, dtype=np.float32)


if __name__ == "__main__":
    rng = np.random.default_rng(0)
    fake = {
        "node_feats": rng.standard_normal((N_NODES, 512), dtype=np.float32),
        "edge_feats": rng.standard_normal((131072, 16), dtype=np.float32),
        "edge_indices": rng.integers(0, N_NODES, (2, 131072)).astype(np.int32),
        "adj": np.maximum(
            (rng.random((N_NODES, N_NODES)) < 0.01).astype(np.float32),
            np.eye(N_NODES, dtype=np.float32),
        ),
    }
    for i, (fin, fout) in enumerate(LAYERS):
        fake[f"W{i}"] = (rng.standard_normal((H, fin, fout)) * 0.05).astype(np.float32)
        fake[f"a{i}"] = (rng.standard_normal((H, 2, fout)) * 0.05).astype(np.float32)
    o = kernel(**fake)
    print("kernel output", o.shape, o.dtype, np.abs(o).mean())



# revision 4
# speedup vs baseline: 1.2071x; 1.2071x over previous
"""Dense 3-layer GAT on 8 TRN2 NeuronCores.

Sharding: each core owns 512 query nodes (rows of the attention score
matrix). Per layer, each core computes h = x @ W and f = x @ (W @ a) for
its own nodes, AllGathers h (bf16) and f (f32) across the 8 cores, then
computes its 512-query slab of masked-softmax attention and the attended
output.

Everything on device is kept transposed (features on partitions, nodes
on the free dim) so layer outputs feed the next layer's matmuls with no
transposes. The whole x/h path runs in bf16; PSUM accumulation is fp32.
Score generation (leaky-relu + exp) is split between the Scalar (ACT)
and Vector (DVE) engines to balance their load; score tiles are deeply
buffered so the elementwise engines run ahead of the PE through the
h AllGathers (which also keeps the PE HAM throttle warm).

kernel(**inputs) takes the full unsharded inputs and returns the full
[4096, 256] output.
"""

from contextlib import ExitStack

import numpy as np
import ml_dtypes

import concourse.mybir as mybir
import concourse.tile as tile
from concourse import bacc
from concourse.bass_utils import run_bass_kernel_spmd
from concourse.masks import make_identity

P = 128
N_NODES = 4096
S = 512                    # nodes per core
NB = N_NODES // P          # 32 global key blocks
H = 4
LAYERS = [(512, 512), (2048, 512), (2048, 64)]
F32 = mybir.dt.float32
BF16 = mybir.dt.bfloat16
AF = mybir.ActivationFunctionType
ALU = mybir.AluOpType

# of every 5 score blocks, this many take the DVE (add+lrelu) path
DVE_OF_5 = (2, 2, 2)

_CACHE = {}


def _build():
    nc = bacc.Bacc("TRN2", target_bir_lowering=False, debug=False, num_devices=8)

    xT0_d = nc.dram_tensor("xT0", [512, S], BF16, kind="ExternalInput")
    adjT_d = nc.dram_tensor("adjT", [N_NODES, S], BF16, kind="ExternalInput")
    W_d = []
    WA_d = []
    for li, (fin, fout) in enumerate(LAYERS):
        W_d.append(nc.dram_tensor(f"W{li}", [fin, H * fout], BF16, kind="ExternalInput"))
        WA_d.append(nc.dram_tensor(f"WA{li}", [fin, 2 * H], BF16, kind="ExternalInput"))
    outT_d = nc.dram_tensor("outT", [H * 64, S], F32, kind="ExternalOutput")

    with tile.TileContext(nc) as tc:
        with ExitStack() as ctx:
            constp = ctx.enter_context(tc.tile_pool(name="const", bufs=1))
            adjp = ctx.enter_context(tc.tile_pool(name="adjp", bufs=1))
            xbp = ctx.enter_context(tc.tile_pool(name="xb", bufs=21))
            xtp = ctx.enter_context(tc.tile_pool(name="xt", bufs=6))
            wtp = ctx.enter_context(tc.tile_pool(name="wt", bufs=17))
            wap = ctx.enter_context(tc.tile_pool(name="wap", bufs=17))
            hfp = ctx.enter_context(tc.tile_pool(name="hfp", bufs=3))
            hgp = ctx.enter_context(tc.tile_pool(name="hgp", bufs=3))
            scp = ctx.enter_context(tc.tile_pool(name="scp", bufs=4))
            stp = ctx.enter_context(tc.tile_pool(name="stp", bufs=10))
            zp = ctx.enter_context(tc.tile_pool(name="zp", bufs=3))
            fbp = ctx.enter_context(tc.tile_pool(name="fbp", bufs=1))
            fdp = ctx.enter_context(tc.tile_pool(name="fdp", bufs=2))
            evp = ctx.enter_context(tc.tile_pool(name="evp", bufs=3))
            rcp = ctx.enter_context(tc.tile_pool(name="rcp", bufs=2))
            psO = ctx.enter_context(tc.tile_pool(name="psO", bufs=5, space="PSUM"))
            psH = ctx.enter_context(tc.tile_pool(name="psH", bufs=2, space="PSUM"))
            psR = ctx.enter_context(tc.tile_pool(name="psR", bufs=1, space="PSUM"))
            dr = ctx.enter_context(tc.tile_pool(name="dram", bufs=1, space="DRAM"))

            ident = constp.tile([P, P], F32, tag="ident")
            make_identity(nc, ident[:])
            # dummy collective to absorb ncfw first-call staging cost while
            # the layer-0 matmuls run
            warm_in = dr.tile([P, 4], F32, tag="warm_in")
            warm_out = dr.tile([8, P, 4], F32, tag="warm_out", addr_space="Shared")
            warm_sb = constp.tile([P, 4], F32, tag="warm_sb", name="warm_sb")
            nc.any.memset(warm_sb[:], 0.0)
            nc.sync.dma_start(warm_in[:], warm_sb[:])
            nc.gpsimd.collective_compute(
                "AllGather", ALU.bypass,
                replica_groups=[list(range(8))],
                ins=[warm_in[:].opt()], outs=[warm_out[:].opt()],
            )
            ones_r = constp.tile([1, P], F32, tag="ones_r")
            nc.any.memset(ones_r[:], 1.0)
            ones_c = constp.tile([P, 1], BF16, tag="ones_c")
            nc.any.memset(ones_c[:], 1.0)

            # resident adjacency (transposed slab), bf16, [key m, own query n]
            adjT_res = adjp.tile([P, NB, S], BF16, tag="adjT")
            nc.sync.dma_start(
                adjT_res[:], adjT_d[:].rearrange("(nb p) n -> p nb n", p=P)
            )

            # layer-0 x^T (own nodes), bf16
            xb_cur = []
            for kb in range(4):
                tb = xbp.tile([P, S], BF16, tag="xb")
                nc.sync.dma_start(tb[:], xT0_d[kb * P:(kb + 1) * P, :])
                xb_cur.append(tb)

            for li, (fin, fout) in enumerate(LAYERS):
                KB = fin // P
                agh_in = dr.tile([H, S, fout], BF16, tag=f"aghi{li}")
                if li < 2:
                    agh_out = [
                        dr.tile([8, 2, S, fout], BF16, tag=f"agho{li}_{pp}",
                                name=f"agho{li}_{pp}", addr_space="Shared")
                        for pp in range(2)
                    ]
                else:
                    agh_out = [dr.tile([8, H, S, fout], BF16, tag=f"agho{li}",
                                       name=f"agho{li}", addr_space="Shared")]
                agf_in = dr.tile([2 * H, S], F32, tag=f"agfi{li}")
                agf_out = dr.tile([8, 2 * H, S], F32, tag=f"agfo{li}",
                                  addr_space="Shared")

                # ---- phase A1: f^T = WA^T @ x^T (bf16 in, f32 psum) ----
                wa_tiles = []
                for kb in range(KB):
                    t = wap.tile([P, 2 * H], BF16, tag="wa")
                    nc.sync.dma_start(t[:], WA_d[li][kb * P:(kb + 1) * P, :])
                    wa_tiles.append(t)
                pfT = psH.tile([2 * H, S], F32, tag="ph", name=f"pfT{li}")
                for kb in range(KB):
                    nc.tensor.matmul(
                        pfT[:], wa_tiles[kb][:], xb_cur[kb][:],
                        start=(kb == 0), stop=(kb == KB - 1),
                    )
                fT_loc = hfp.tile([2 * H, S], F32, tag="ftl")
                nc.scalar.copy(fT_loc[:], pfT[:])
                nc.sync.dma_start(agf_in[:], fT_loc[:])

                # f gather is tiny; issue it before the h matmuls so it hides
                nc.gpsimd.collective_compute(
                    "AllGather", ALU.bypass,
                    replica_groups=[list(range(8))],
                    ins=[agf_in[:].opt()], outs=[agf_out[:].opt()],
                )

                # ---- phase A2: h = x @ W (bf16) ----
                if li < 2:
                    w_tiles = {}
                    for h in range(H):
                        for kb in range(KB):
                            t = wtp.tile([P, fout], BF16, tag="wt",
                                         name=f"w{li}_{h}_{kb}")
                            nc.sync.dma_start(
                                t[:],
                                W_d[li][kb * P:(kb + 1) * P,
                                        h * fout:(h + 1) * fout],
                            )
                            w_tiles[(h, kb)] = t
                    for h in range(H):
                        for b in range(4):
                            ph = psH.tile([P, S], F32, tag="ph",
                                          name=f"ph{li}_{h}_{b}")
                            for kb in range(KB):
                                nc.tensor.matmul(
                                    ph[:, 0:fout],
                                    xb_cur[kb][:, b * P:(b + 1) * P],
                                    w_tiles[(h, kb)][:],
                                    start=(kb == 0), stop=(kb == KB - 1),
                                )
                            h_sb = hfp.tile([P, fout], BF16, tag="hsb")
                            nc.vector.tensor_copy(h_sb[:], ph[:, 0:fout])
                            nc.gpsimd.dma_start(
                                agh_in[h, b * P:(b + 1) * P, :], h_sb[:]
                            )
                        # per-head-pair gather overlaps the next heads' matmuls
                        if h % 2 == 1:
                            nc.gpsimd.collective_compute(
                                "AllGather", ALU.bypass,
                                replica_groups=[list(range(8))],
                                ins=[agh_in[h - 1:h + 1].opt()],
                                outs=[agh_out[h // 2][:].opt()],
                            )
                else:
                    # merged: one rhs covers all 4 heads (4*64 = 256 cols)
                    w_tiles2 = []
                    for kb in range(KB):
                        t = wtp.tile([P, H * fout], BF16, tag="wt",
                                     name=f"w2_{kb}")
                        nc.sync.dma_start(
                            t[:], W_d[li][kb * P:(kb + 1) * P, :]
                        )
                        w_tiles2.append(t)
                    for b in range(4):
                        ph = psH.tile([P, H * fout], F32, tag="ph",
                                      name=f"ph{li}_{b}")
                        for kb in range(KB):
                            nc.tensor.matmul(
                                ph[:, 0:H * fout],
                                xb_cur[kb][:, b * P:(b + 1) * P],
                                w_tiles2[kb][:],
                                start=(kb == 0), stop=(kb == KB - 1),
                            )
                        h_sb = hfp.tile([P, H * fout], BF16, tag="hsb")
                        nc.vector.tensor_copy(h_sb[:], ph[:, 0:H * fout])
                        for h in range(H):
                            nc.gpsimd.dma_start(
                                agh_in[h, b * P:(b + 1) * P, :],
                                h_sb[:, h * fout:(h + 1) * fout],
                            )
                    nc.gpsimd.collective_compute(
                        "AllGather", ALU.bypass,
                        replica_groups=[list(range(8))],
                        ins=[agh_in[:].opt()], outs=[agh_out[0][:].opt()],
                    )

                # ---- phase C: f_src broadcast + f_dst layout ----
                fsb_bcast = fbp.tile([P, H, S], F32, tag="fsb_b")
                for h in range(H):
                    fr = hfp.tile([1, S], F32, tag="fr")
                    nc.sync.dma_start(fr[:], fT_loc[2 * h:2 * h + 1, :])
                    pb = psH.tile([P, S], F32, tag="ph", name=f"pb{li}_{h}")
                    nc.tensor.matmul(pb[:], ones_r[:], fr[:],
                                     start=True, stop=True)
                    nc.vector.tensor_copy(fsb_bcast[:, h, :], pb[:])
                fgat = fdp.tile([64, S], F32, tag="fgat")
                nc.sync.dma_start(fgat[:], agf_out[:].rearrange("r j m -> (r j) m"))
                ptd = psH.tile([P, S], F32, tag="ph", name=f"ptd{li}")
                for c in range(4):
                    nc.tensor.transpose(
                        ptd[:, c * 64:(c + 1) * 64],
                        fgat[:, c * P:(c + 1) * P],
                        ident[0:64, 0:64],
                    )
                # fT_sb[ml, mh, r, j] = f[j][r*512 + mh*128 + ml]
                fT_sb = fdp.tile([P, 4, 8, 2 * H], F32, tag="fdst")
                nc.scalar.copy(
                    fT_sb[:], ptd[:, 0:256].rearrange("p (mh rj) -> p mh rj", mh=4)
                    .rearrange("p mh (r j) -> p mh r j", r=8)
                )

                # ---- phase D: attention ----
                xt_next = []
                xb_next = []
                nob = 1 if fout == 64 else 4
                rows = 64 if fout == 64 else P
                dve5 = DVE_OF_5[li]
                for h in range(H):
                    po = [
                        psO.tile([P, S], F32, tag="po", name=f"po{li}_{h}_{ob}")
                        for ob in range(nob)
                    ]
                    prs = psR.tile([1, S], F32, tag="prs", name=f"prs{li}_{h}")
                    for r in range(8):
                        hg = hgp.tile([P, 4, fout], BF16, tag="hg")
                        if li < 2:
                            src = agh_out[h // 2][r, h % 2]
                        else:
                            src = agh_out[0][r, h]
                        nc.gpsimd.dma_start(
                            hg[:], src.rearrange("(b p) f -> p b f", p=P)
                        )
                        for i2 in range(2):
                            lr4 = scp.tile([P, 2, S], F32, tag="lr")
                            for i in range(2):
                                mb = 4 * r + 2 * i2 + i
                                bias_ap = fT_sb[:, mb % 4, r, 2 * h + 1:2 * h + 2]
                                if mb % 5 < dve5:
                                    z = zp.tile([P, S], F32, tag="z")
                                    nc.vector.tensor_scalar(
                                        z[:], fsb_bcast[:, h, :], bias_ap, None,
                                        ALU.add,
                                    )
                                    nc.vector.scalar_tensor_tensor(
                                        lr4[:, i, :], z[:], 0.2, z[:],
                                        ALU.mult, ALU.max,
                                    )
                                else:
                                    nc.scalar.activation(
                                        lr4[:, i, :], fsb_bcast[:, h, :], AF.Prelu,
                                        bias=bias_ap, scale=1.0, alpha=0.2,
                                    )
                            ex4 = scp.tile([P, 2, S], BF16, tag="ex")
                            nc.scalar.activation(ex4[:], lr4[:], AF.Exp,
                                                 bias=0.0, scale=1.0)
                            st4 = stp.tile([P, 2, S], BF16, tag="st")
                            nc.vector.tensor_tensor(
                                st4[:], ex4[:],
                                adjT_res[:, 4 * r + 2 * i2:4 * r + 2 * i2 + 2, :],
                                ALU.mult,
                            )
                            for i in range(2):
                                mb = 4 * r + 2 * i2 + i
                                s_t = st4[:, i, :]
                                for ob in range(nob):
                                    nc.tensor.matmul(
                                        po[ob][0:rows, :],
                                        hg[:, 2 * i2 + i, ob * P:(ob + 1) * P]
                                        if fout != 64 else hg[:, 2 * i2 + i, :],
                                        s_t,
                                        start=(mb == 0), stop=(mb == NB - 1),
                                    )
                                nc.tensor.matmul(
                                    prs[:], ones_c[:], s_t,
                                    start=(mb == 0), stop=(mb == NB - 1),
                                )

                    # normalize + ELU; PSUM is read directly by DVE
                    rsum = rcp.tile([1, S], F32, tag="rsum")
                    nc.vector.tensor_copy(rsum[:], prs[:])
                    pb2 = psH.tile([P, S], F32, tag="ph", name=f"pb2{li}_{h}")
                    nc.tensor.matmul(pb2[:], ones_r[:], rsum[:], start=True, stop=True)
                    rb = rcp.tile([P, S], F32, tag="rb")
                    nc.vector.reciprocal_approx_fast(rb[:], pb2[:])
                    for ob in range(nob):
                        t0 = evp.tile([rows, S], F32, tag="t0")
                        nc.vector.tensor_tensor(t0[:], po[ob][0:rows, :],
                                                rb[0:rows, :], ALU.mult)
                        # elu(x) = min(exp(x) - 1, relu(x))
                        em = evp.tile([rows, S], F32, tag="em")
                        nc.scalar.activation(em[:], t0[:], AF.Exp, bias=0.0, scale=1.0)
                        rl = evp.tile([rows, S], F32, tag="rl")
                        nc.vector.tensor_scalar_max(rl[:], t0[:], 0.0)
                        if li < 2:
                            xbn = xbp.tile([rows, S], BF16, tag="xb", name=f"xb{li}")
                            nc.vector.scalar_tensor_tensor(
                                xbn[:], em[:], -1.0, rl[:], ALU.add, ALU.min
                            )
                            xb_next.append(xbn)
                        else:
                            xnt = evp.tile([rows, S], F32, tag="xn", name=f"xn{li}")
                            nc.vector.scalar_tensor_tensor(
                                xnt[:], em[:], -1.0, rl[:], ALU.add, ALU.min
                            )
                            em2 = evp.tile([rows, S], F32, tag="em", name=f"em2{li}")
                            nc.scalar.activation(em2[:], xnt[:], AF.Exp,
                                                 bias=0.0, scale=1.0)
                            rl2 = evp.tile([rows, S], F32, tag="rl", name=f"rl2{li}")
                            nc.vector.tensor_scalar_max(rl2[:], xnt[:], 0.0)
                            x2 = xtp.tile([rows, S], F32, tag="xt", name=f"x2{li}")
                            nc.vector.scalar_tensor_tensor(
                                x2[:], em2[:], -1.0, rl2[:], ALU.add, ALU.min
                            )
                            xt_next.append(x2)

                xb_cur = xb_next

            # final output: xt_next is 4 tiles of [64, 512] (head-major)
            for h in range(H):
                nc.sync.dma_start(outT_d[h * 64:(h + 1) * 64, :], xt_next[h][:])

    nc.compile()
    return nc


def build_in_maps(inputs):
    node_feats = np.ascontiguousarray(inputs["node_feats"], dtype=np.float32)
    adj = np.asarray(inputs["adj"], dtype=np.float32)
    Ws = [np.asarray(inputs[f"W{i}"], dtype=np.float32) for i in range(3)]
    As = [np.asarray(inputs[f"a{i}"], dtype=np.float32) for i in range(3)]

    WAs = []
    Wcats = []
    for W, a in zip(Ws, As):
        wa = np.einsum(
            "hfo,hjo->fhj", W.astype(np.float64), a.astype(np.float64)
        ).reshape(W.shape[1], 2 * H).astype(ml_dtypes.bfloat16)
        WAs.append(np.ascontiguousarray(wa))
        wcat = np.ascontiguousarray(
            np.transpose(W, (1, 0, 2)).reshape(W.shape[1], -1)
        ).astype(ml_dtypes.bfloat16)
        Wcats.append(wcat)

    in_maps = []
    for c in range(8):
        rows = slice(c * S, (c + 1) * S)
        m = {
            "xT0": np.ascontiguousarray(node_feats[rows].T).astype(ml_dtypes.bfloat16),
            "adjT": np.ascontiguousarray(adj[rows].T).astype(ml_dtypes.bfloat16),
        }
        for i in range(3):
            m[f"W{i}"] = Wcats[i]
            m[f"WA{i}"] = WAs[i]
        in_maps.append(m)
    return in_maps


def kernel(**inputs):
    if "nc" not in _CACHE:
        _CACHE["nc"] = _build()
    nc = _CACHE["nc"]
    in_maps = build_in_maps(inputs)
    res = run_bass_kernel_spmd(nc, in_maps, core_ids=list(range(8)))
    out = np.concatenate([r["outT"].T for r in res.results], axis=0)
    return np.ascontiguousarray(out, dtype=np.float32)


if __name__ == "__main__":
    rng = np.random.default_rng(0)
    fake = {
        "node_feats": rng.standard_normal((N_NODES, 512), dtype=np.float32),
        "edge_feats": rng.standard_normal((131072, 16), dtype=np.float32),
        "edge_indices": rng.integers(0, N_NODES, (2, 131072)).astype(np.int32),
        "adj": np.maximum(
            (rng.random((N_NODES, N_NODES)) < 0.01).astype(np.float32),
            np.eye(N_NODES, dtype=np.float32),
        ),
    }
    for i, (fin, fout) in enumerate(LAYERS):
        fake[f"W{i}"] = (rng.standard_normal((H, fin, fout)) * 0.05).astype(np.float32)
        fake[f"a{i}"] = (rng.standard_normal((H, 2, fout)) * 0.05).astype(np.float32)
    o = kernel(**fake)
    print("kernel output", o.shape, o.dtype, np.abs(o).mean())
